# revision 35
# baseline (speedup 1.0000x reference)
"""Trainium2 Bass kernel for banded local attention (kernel_size=128).

Problem: x[4,4096,512]; q = x@Wq.T+bq, k = x@Wk.T+bk (H=512);
scores = q@k.T masked to |i-j|<128; softmax; out = attn @ x.

Sharding: 8 cores = 4 batches x 2 sequence halves (2048 queries each) with a
128-row halo of keys on each side (2304 local key rows, zero padded at the
global sequence edges). For the h=1 half the sequence is passed REVERSED so
the padded/invalid key region is always local rows [0,128) and the edge mask
is only needed for query block 0 -> all 8 cores run the identical program
(pure SPMD, no collectives). Host un-reverses the h=1 outputs.

Key algebraic fold (v4): s_ij = q_i.k_j = x_i^T (Wq^T Wk) x_j + (per-i const)
+ (Wk^T bq).x_j + (const). The per-i and const terms are softmax-invariant
and dropped; M = Wq^T Wk is folded on the host so the device projects ONLY
g = M x (the q-projection disappears entirely -- scores use raw xT, already
resident, as lhsT); beta_j = (Wk^T bq).x_j is added into the scores PSUM by
an all-ones matmul against a host-precomputed beta/128 tile.

Precision: the score spread (sigma ~ 22) makes the softmax near-argmax, so
x / M / g stay f32r (fp22); p / values / out are bf16 (p errors cancel via
the l normalizer; bf16 V adds ~0.2% per element).

Datapath per 128-query block:
  PSUM scores group: 4 f32r matmuls (xT-block^T @ gT window) + identity
  matmul adding the on-chip band mask + ones matmul adding beta. Then DVE
  negated rowmax -> ACT exp (bf16 p, row-sum l via accum_out; invalid
  positions exp(-1e30)=0) -> PE transposes (bf16 1 cyc/row) -> ACT cast to
  SBUF -> 3 bf16 matmuls against resident bf16 values -> DVE 1/l scale ->
  bf16 out DMA (host widens).
"""
import sys

if "/opt/trn_rl_repo" not in sys.path:
    sys.path.insert(0, "/opt/trn_rl_repo")

import numpy as np

B, S, D, H = 4, 4096, 512, 512
KS = 128
HALF = S // 2            # 2048 queries per core
HALO = KS                # 128
SK = HALF + 2 * HALO     # 2304 local key rows
KT = SK // 128           # 18 key row tiles
WIN = 3 * 128            # 384-wide key window per query block
NBLK = HALF // 128       # 16 query blocks
NEG = -1e30
N_CORES = 8

DT = D // 128   # 4 d-tiles

# g-projection chunks over SK columns (= xT chunk tiles); first chunk small
# so the first matmul starts as early as possible (f32r needs >= 256 free);
# last chunks small so the final attention blocks start earlier
CHUNKS = [(0, 256), (256, 512), (768, 512), (1280, 512), (1792, 256),
          (2048, 256)]
# attention blocks emitted once g covers the block's window
CHUNK_BLOCKS = [[], [0, 1, 2, 3], [4, 5, 6, 7], [8, 9, 10, 11], [12, 13],
                [14, 15]]

_cached = {}


def _build_program():
    import concourse.bass as bass
    import concourse.tile as tile
    import concourse.mybir as mybir
    from concourse import bacc
    from concourse.masks import make_identity

    f32 = mybir.dt.float32
    f32r = mybir.dt.float32r
    bf16 = mybir.dt.bfloat16
    AF = mybir.ActivationFunctionType
    AX = mybir.AxisListType

    nc = bacc.Bacc("TRN2", target_bir_lowering=False, debug=False,
                   num_devices=N_CORES)

    # dram inputs (host pre-arranged; see kernel())
    mt_d = nc.dram_tensor("mtp", [128, DT, D], f32r,
                          kind="ExternalInput").ap()
    # one dram tensor per x chunk (contiguous per partition -> one DMA
    # descriptor per partition row instead of DT)
    xT_ds = [nc.dram_tensor(f"xTp{c}", [128, DT, cw], f32r,
                            kind="ExternalInput").ap()
             for c, (c0, cw) in enumerate(CHUNKS)]
    beta_d = nc.dram_tensor("betap", [128, SK], bf16,
                            kind="ExternalInput").ap()
    xrow_d = nc.dram_tensor("xrowp", [128, KT, D], bf16,
                            kind="ExternalInput").ap()
    out_d = nc.dram_tensor("out", [HALF, D], bf16, kind="ExternalOutput").ap()

    with tile.TileContext(nc) as tc:
        with (
            tc.tile_pool(name="big", bufs=1) as big,
            tc.tile_pool(name="work", bufs=3) as work,
            tc.tile_pool(name="stat", bufs=6) as stat,
            tc.tile_pool(name="psP", bufs=2, space="PSUM") as psP,
            tc.tile_pool(name="psS", bufs=2, space="PSUM") as psS,
            tc.tile_pool(name="psT", bufs=2, space="PSUM") as psT,
            tc.tile_pool(name="psO", bufs=2, space="PSUM") as psO,
        ):
            # ---- input DMAs, ordered so g-proj chunk 0 starts earliest:
            # M arrives in contraction-tile slices so the first accumulation
            # member only waits for slice 0 + the small first x chunk
            mt = [big.tile([128, D], f32r, tag=f"mt{i}", name=f"mt{i}")
                  for i in range(DT)]
            xTc = [big.tile([128, DT, cw], f32r, tag=f"xT{c}", name=f"xT{c}")
                   for c, (c0, cw) in enumerate(CHUNKS)]
            nc.sync.dma_start(mt[0], mt_d[:, 0, :])
            nc.sync.dma_start(xTc[0], xT_ds[0])
            for dti in range(1, DT):
                nc.sync.dma_start(mt[dti], mt_d[:, dti, :])
            nc.sync.dma_start(xTc[1], xT_ds[1])
            beta = big.tile([128, SK], bf16, tag="beta", name="beta")
            nc.sync.dma_start(beta, beta_d)
            nc.sync.dma_start(xTc[2], xT_ds[2])
            xrowp = big.tile([128, KT, D], bf16, tag="xrowp", name="xrowp")
            nc.sync.dma_start(xrowp, xrow_d)
            for c in (3, 4, 5):
                nc.sync.dma_start(xTc[c], xT_ds[c])

            # ---- on-chip constants (GpSimd; overlaps the DMAs) ----
            ident = big.tile([128, 128], bf16, tag="ident")
            make_identity(nc, ident)
            # PE p-state warm-up: ~30 standalone bf16 LDWEIGHTS while the
            # first input DMAs are still in flight, so the real matmuls run
            # at full clock from the start (the PE ramps over ~3us of
            # activity; these cost nothing on an otherwise idle engine)
            for _ in range(30):
                nc.tensor.ldweights(ident)
            # additive band masks over the 384-wide window: tile 0 valid iff
            # y > a, tile 1 always valid, tile 2 valid iff y < a; the edge
            # variant (block 0) masks all of tile 0 (padded halo rows)
            masks = []
            for mi in range(2):
                mk = big.tile([128, WIN], bf16, tag=f"mask{mi}")
                nc.gpsimd.memset(mk, 0.0)
                if mi == 0:
                    nc.gpsimd.affine_select(
                        out=mk[:, 0:128], in_=mk[:, 0:128],
                        compare_op=mybir.AluOpType.is_ge, fill=NEG,
                        base=-1, pattern=[[1, 128]], channel_multiplier=-1)
                else:
                    nc.gpsimd.memset(mk[:, 0:128], NEG)
                nc.gpsimd.affine_select(
                    out=mk[:, 256:384], in_=mk[:, 256:384],
                    compare_op=mybir.AluOpType.is_ge, fill=NEG,
                    base=-1, pattern=[[-1, 128]], channel_multiplier=1)
                masks.append(mk)

            # ---- g projection: gT[dto][d, j] = sum_dti M x ----
            gT = [big.tile([128, SK], f32r, tag=f"gT{t}", name=f"gT{t}")
                  for t in range(DT)]
            ncopy = [0]

            def proj_chunk(c0, cw, xc):
                for dto in range(DT):
                    ps = psP.tile([128, 512], f32, tag="proj")
                    for dti in range(DT):
                        nc.tensor.matmul(
                            ps[:, :cw],
                            lhsT=mt[dti][:, dto * 128:(dto + 1) * 128],
                            rhs=xTc[xc][:, dti, 0:cw],
                            start=(dti == 0),
                            stop=(dti == DT - 1),
                        )
                    # alternate the PSUM->SBUF cast between ACT and DVE
                    if ncopy[0] % 2 == 0:
                        nc.scalar.activation(
                            gT[dto][:, c0:c0 + cw], ps[:, :cw],
                            AF.Identity, bias=0.0, scale=1.0)
                    else:
                        nc.vector.tensor_copy(
                            gT[dto][:, c0:c0 + cw], ps[:, :cw])
                    ncopy[0] += 1

            # ---- per query-block attention ----
            def attention(qb):
                j0 = qb * 128
                # query cols in xT: [HALO + j0, HALO + j0 + 128)
                qc = HALO + j0
                xc = next(c for c, (c0, cw) in enumerate(CHUNKS)
                          if c0 <= qc and qc + 128 <= c0 + cw)
                c0, cw = CHUNKS[xc]
                qoff = qc - c0
                # GpSimd (otherwise idle) combines band mask + beta window
                comb = work.tile([128, WIN], bf16, tag="comb")
                nc.gpsimd.tensor_add(
                    comb, masks[1] if qb == 0 else masks[0],
                    beta[:, j0:j0 + WIN])
                s_ps = psS.tile([128, WIN], f32, tag="s")
                for dt_i in range(DT):
                    nc.tensor.matmul(
                        s_ps,
                        lhsT=xTc[xc][:, dt_i, qoff:qoff + 128],
                        rhs=gT[dt_i][:, j0:j0 + WIN],
                        start=(dt_i == 0),
                        stop=False,
                    )
                # mask+beta added into the PSUM group via identity matmul
                nc.tensor.matmul(
                    s_ps, lhsT=ident, rhs=comb, start=False, stop=True)

                negm = stat.tile([128, 1], f32, tag="negm")
                nc.vector.reduce_max(negm, s_ps, axis=AX.X, negate=True)
                p = work.tile([128, WIN], bf16, tag="p")
                lsum = stat.tile([128, 1], f32, tag="lsum")
                nc.scalar.activation(p, s_ps, AF.Exp, bias=negm, scale=1.0,
                                     accum_out=lsum)
                rinv = stat.tile([128, 1], f32, tag="rinv")
                nc.vector.reciprocal(rinv, lsum)

                # transpose p (bf16, 1 cyc/row); ACT copies PSUM -> SBUF
                pT_ps = psT.tile([128, 3, 128], bf16, tag="pT")
                for jt in range(3):
                    nc.tensor.transpose(
                        pT_ps[:, jt, :], p[:, jt * 128:(jt + 1) * 128], ident)
                pT = work.tile([128, 3, 128], bf16, tag="pTs")
                nc.scalar.activation(pT, pT_ps, AF.Identity, bias=0.0,
                                     scale=1.0)

                # out_blk[i, d] = sum_j p[i, j] xrow[j, d]
                o_ps = psO.tile([128, D], f32, tag="o")
                for jt in range(3):
                    nc.tensor.matmul(o_ps, lhsT=pT[:, jt, :],
                                     rhs=xrowp[:, qb + jt, :],
                                     start=(jt == 0), stop=(jt == 2))
                o_sb = work.tile([128, D], bf16, tag="o_sb")
                nc.vector.tensor_scalar_mul(o_sb, o_ps, rinv)
                nc.sync.dma_start(out_d[qb * 128:(qb + 1) * 128, :], o_sb)

            for c, (c0, cw) in enumerate(CHUNKS):
                proj_chunk(c0, cw, c)
                for qb in CHUNK_BLOCKS[c]:
                    attention(qb)

    nc.compile()
    return nc


def _get_program():
    if "nc" not in _cached:
        _cached["nc"] = _build_program()
    return _cached["nc"]


def kernel(x, Wq_w, Wq_b, Wk_w, Wk_b, _trace=False):
    import ml_dtypes
    from concourse.bass_utils import run_bass_kernel_spmd

    bf16 = ml_dtypes.bfloat16

    x = np.ascontiguousarray(np.asarray(x, np.float32))
    Wq_w = np.asarray(Wq_w, np.float64)
    Wk_w = np.asarray(Wk_w, np.float64)
    # fold the projections: s_ij = x_i^T M x_j + beta_j (+ softmax-invariant
    # terms); M = Wq^T Wk, v = Wk^T bq
    M = (Wq_w.T @ Wk_w).astype(np.float32)
    v = (Wk_w.T @ np.asarray(Wq_b, np.float64)).astype(np.float32)
    # mtp[p, dti, m] = M[m, dti*128+p]
    mtp = np.ascontiguousarray(
        M.T.reshape(DT, 128, D).transpose(1, 0, 2))

    nc = _get_program()

    in_maps = []
    for core in range(N_CORES):
        b, h = divmod(core, 2)
        x_halo = np.zeros((SK, D), np.float32)
        if h == 0:
            x_halo[HALO:] = x[b, 0:HALF + HALO]
        else:
            x_halo[HALO:] = x[b, S - HALF - HALO:][::-1]
        # xTp[p, dt, c] = x_halo[c, dt*128+p], split into chunk tensors
        xTp = x_halo.T.reshape(DT, 128, SK).transpose(1, 0, 2)
        # xrowp[p, kt, d] = x_halo[kt*128+p, d]
        xrp = np.ascontiguousarray(
            x_halo.reshape(KT, 128, D).transpose(1, 0, 2)).astype(bf16)
        betar = (x_halo @ v).astype(bf16)
        im = {
            "mtp": mtp,
            "betap": np.ascontiguousarray(
                np.broadcast_to(betar[None, :], (128, SK))),
            "xrowp": xrp,
        }
        for c, (c0, cw) in enumerate(CHUNKS):
            im[f"xTp{c}"] = np.ascontiguousarray(xTp[:, :, c0:c0 + cw])
        in_maps.append(im)

    res = run_bass_kernel_spmd(nc, in_maps, core_ids=list(range(N_CORES)),
                               trace=_trace)
    _cached["last_result"] = res

    y = np.zeros((B, S, D), np.float32)
    for core in range(N_CORES):
        b, h = divmod(core, 2)
        o = res.results[core]["out"].astype(np.float32)
        if h == 0:
            y[b, :HALF] = o
        else:
            y[b, HALF:] = o[::-1]
    return y


# revision 39
# speedup vs baseline: 1.0166x; 1.0166x over previous
"""Trainium2 Bass kernel for banded local attention (kernel_size=128).

Problem: x[4,4096,512]; q = x@Wq.T+bq, k = x@Wk.T+bk (H=512);
scores = q@k.T masked to |i-j|<128; softmax; out = attn @ x.

Sharding: 8 cores = 4 batches x 2 sequence halves (2048 queries each) with a
128-row halo of keys on each side (2304 local key rows, zero padded at the
global sequence edges). For the h=1 half the sequence is passed REVERSED so
the padded/invalid key region is always local rows [0,128) and the edge mask
is only needed for query block 0 -> all 8 cores run the identical program
(pure SPMD, no collectives). Host un-reverses the h=1 outputs.

Key algebraic fold (v4): s_ij = q_i.k_j = x_i^T (Wq^T Wk) x_j + (per-i const)
+ (Wk^T bq).x_j + (const). The per-i and const terms are softmax-invariant
and dropped; M = Wq^T Wk is folded on the host so the device projects ONLY
g = M x (the q-projection disappears entirely -- scores use raw xT, already
resident, as lhsT); beta_j = (Wk^T bq).x_j is added into the scores PSUM by
an all-ones matmul against a host-precomputed beta/128 tile.

Precision: the score spread (sigma ~ 22) makes the softmax near-argmax, so
x / M / g stay f32r (fp22); p / values / out are bf16 (p errors cancel via
the l normalizer; bf16 V adds ~0.2% per element).

Datapath per 128-query block:
  PSUM scores group: 4 f32r matmuls (xT-block^T @ gT window) + identity
  matmul adding the on-chip band mask + ones matmul adding beta. Then DVE
  negated rowmax -> ACT exp (bf16 p, row-sum l via accum_out; invalid
  positions exp(-1e30)=0) -> PE transposes (bf16 1 cyc/row) -> ACT cast to
  SBUF -> 3 bf16 matmuls against resident bf16 values -> DVE 1/l scale ->
  bf16 out DMA (host widens).
"""
import sys

if "/opt/trn_rl_repo" not in sys.path:
    sys.path.insert(0, "/opt/trn_rl_repo")

import numpy as np

B, S, D, H = 4, 4096, 512, 512
KS = 128
HALF = S // 2            # 2048 queries per core
HALO = KS                # 128
SK = HALF + 2 * HALO     # 2304 local key rows
KT = SK // 128           # 18 key row tiles
WIN = 3 * 128            # 384-wide key window per query block
NBLK = HALF // 128       # 16 query blocks
NEG = -1e30
N_CORES = 8

DT = D // 128   # 4 d-tiles

# g-projection chunks over SK columns (= xT chunk tiles); first chunk small
# so the first matmul starts as early as possible (f32r needs >= 256 free);
# last chunks small so the final attention blocks start earlier
CHUNKS = [(0, 256), (256, 512), (768, 512), (1280, 512), (1792, 256),
          (2048, 256)]
# attention blocks emitted once g covers the block's window
CHUNK_BLOCKS = [[], [0, 1, 2, 3], [4, 5, 6, 7], [8, 9, 10, 11], [12, 13],
                [14, 15]]

_cached = {}


def _build_program():
    import concourse.bass as bass
    import concourse.tile as tile
    import concourse.mybir as mybir
    from concourse import bacc
    from concourse.masks import make_identity

    f32 = mybir.dt.float32
    f32r = mybir.dt.float32r
    bf16 = mybir.dt.bfloat16
    AF = mybir.ActivationFunctionType
    AX = mybir.AxisListType

    nc = bacc.Bacc("TRN2", target_bir_lowering=False, debug=False,
                   num_devices=N_CORES)

    # dram inputs (host pre-arranged; see kernel())
    mt_d = nc.dram_tensor("mtp", [128, DT, D], f32r,
                          kind="ExternalInput").ap()
    # one dram tensor per x chunk (contiguous per partition -> one DMA
    # descriptor per partition row instead of DT)
    xT_ds = [nc.dram_tensor(f"xTp{c}", [128, DT, cw], f32r,
                            kind="ExternalInput").ap()
             for c, (c0, cw) in enumerate(CHUNKS)]
    beta_d = nc.dram_tensor("betap", [128, SK], bf16,
                            kind="ExternalInput").ap()
    xrow_d = nc.dram_tensor("xrowp", [128, KT, D], bf16,
                            kind="ExternalInput").ap()
    out_d = nc.dram_tensor("out", [HALF, D], bf16, kind="ExternalOutput").ap()

    with tile.TileContext(nc) as tc:
        with (
            tc.tile_pool(name="big", bufs=1) as big,
            tc.tile_pool(name="work", bufs=3) as work,
            tc.tile_pool(name="stat", bufs=6) as stat,
            tc.tile_pool(name="psP", bufs=2, space="PSUM") as psP,
            tc.tile_pool(name="psS", bufs=2, space="PSUM") as psS,
            tc.tile_pool(name="psT", bufs=2, space="PSUM") as psT,
            tc.tile_pool(name="psO", bufs=2, space="PSUM") as psO,
        ):
            # ---- input DMAs, ordered so g-proj chunk 0 starts earliest:
            # M arrives in contraction-tile slices so the first accumulation
            # member only waits for slice 0 + the small first x chunk
            mt = [big.tile([128, D], f32r, tag=f"mt{i}", name=f"mt{i}")
                  for i in range(DT)]
            xTc = [big.tile([128, DT, cw], f32r, tag=f"xT{c}", name=f"xT{c}")
                   for c, (c0, cw) in enumerate(CHUNKS)]
            nc.sync.dma_start(mt[0], mt_d[:, 0, :])
            nc.sync.dma_start(xTc[0], xT_ds[0])
            for dti in range(1, DT):
                nc.sync.dma_start(mt[dti], mt_d[:, dti, :])
            nc.sync.dma_start(xTc[1], xT_ds[1])
            beta = big.tile([128, SK], bf16, tag="beta", name="beta")
            nc.sync.dma_start(beta, beta_d)
            nc.sync.dma_start(xTc[2], xT_ds[2])
            xrowp = big.tile([128, KT, D], bf16, tag="xrowp", name="xrowp")
            nc.sync.dma_start(xrowp, xrow_d)
            for c in (3, 4, 5):
                nc.sync.dma_start(xTc[c], xT_ds[c])

            # ---- on-chip constants (GpSimd; overlaps the DMAs) ----
            ident = big.tile([128, 128], bf16, tag="ident")
            make_identity(nc, ident)
            # PE p-state warm-up: ~30 standalone bf16 LDWEIGHTS while the
            # first input DMAs are still in flight, so the real matmuls run
            # at full clock from the start (the PE ramps over ~3us of
            # activity; these cost nothing on an otherwise idle engine)
            for _ in range(30):
                nc.tensor.ldweights(ident)
            # additive band masks over the 384-wide window: tile 0 valid iff
            # y > a, tile 1 always valid, tile 2 valid iff y < a; the edge
            # variant (block 0) masks all of tile 0 (padded halo rows)
            masks = []
            for mi in range(2):
                mk = big.tile([128, WIN], bf16, tag=f"mask{mi}")
                nc.gpsimd.memset(mk, 0.0)
                if mi == 0:
                    nc.gpsimd.affine_select(
                        out=mk[:, 0:128], in_=mk[:, 0:128],
                        compare_op=mybir.AluOpType.is_ge, fill=NEG,
                        base=-1, pattern=[[1, 128]], channel_multiplier=-1)
                else:
                    nc.gpsimd.memset(mk[:, 0:128], NEG)
                nc.gpsimd.affine_select(
                    out=mk[:, 256:384], in_=mk[:, 256:384],
                    compare_op=mybir.AluOpType.is_ge, fill=NEG,
                    base=-1, pattern=[[-1, 128]], channel_multiplier=1)
                masks.append(mk)

            # ---- g projection: gT[dto][d, j] = sum_dti M x ----
            gT = [big.tile([128, SK], f32r, tag=f"gT{t}", name=f"gT{t}")
                  for t in range(DT)]
            ncopy = [0]

            def proj_chunk(c0, cw, xc):
                for dto in range(DT):
                    ps = psP.tile([128, 512], f32, tag="proj")
                    for dti in range(DT):
                        nc.tensor.matmul(
                            ps[:, :cw],
                            lhsT=mt[dti][:, dto * 128:(dto + 1) * 128],
                            rhs=xTc[xc][:, dti, 0:cw],
                            start=(dti == 0),
                            stop=(dti == DT - 1),
                        )
                    # alternate the PSUM->SBUF cast between ACT and DVE
                    if ncopy[0] % 2 == 0:
                        nc.scalar.activation(
                            gT[dto][:, c0:c0 + cw], ps[:, :cw],
                            AF.Identity, bias=0.0, scale=1.0)
                    else:
                        nc.vector.tensor_copy(
                            gT[dto][:, c0:c0 + cw], ps[:, :cw])
                    ncopy[0] += 1

            # ---- per query-block attention ----
            def attention(qb):
                j0 = qb * 128
                # query cols in xT: [HALO + j0, HALO + j0 + 128)
                qc = HALO + j0
                xc = next(c for c, (c0, cw) in enumerate(CHUNKS)
                          if c0 <= qc and qc + 128 <= c0 + cw)
                c0, cw = CHUNKS[xc]
                qoff = qc - c0
                # GpSimd (otherwise idle) combines band mask + beta window
                comb = work.tile([128, WIN], bf16, tag="comb")
                nc.gpsimd.tensor_add(
                    comb, masks[1] if qb == 0 else masks[0],
                    beta[:, j0:j0 + WIN])
                s_ps = psS.tile([128, WIN], f32, tag="s")
                for dt_i in range(DT):
                    nc.tensor.matmul(
                        s_ps,
                        lhsT=xTc[xc][:, dt_i, qoff:qoff + 128],
                        rhs=gT[dt_i][:, j0:j0 + WIN],
                        start=(dt_i == 0),
                        stop=False,
                    )
                # mask+beta added into the PSUM group via identity matmul
                nc.tensor.matmul(
                    s_ps, lhsT=ident, rhs=comb, start=False, stop=True)

                negm = stat.tile([128, 1], f32, tag="negm")
                nc.vector.reduce_max(negm, s_ps, axis=AX.X, negate=True)
                p = work.tile([128, WIN], bf16, tag="p")
                lsum = stat.tile([128, 1], f32, tag="lsum")
                nc.scalar.activation(p, s_ps, AF.Exp, bias=negm, scale=1.0,
                                     accum_out=lsum)
                rinv = stat.tile([128, 1], f32, tag="rinv")
                nc.vector.reciprocal(rinv, lsum)

                # transpose p (bf16, 1 cyc/row); ACT copies PSUM -> SBUF
                pT_ps = psT.tile([128, 3, 128], bf16, tag="pT")
                for jt in range(3):
                    nc.tensor.transpose(
                        pT_ps[:, jt, :], p[:, jt * 128:(jt + 1) * 128], ident)
                pT = work.tile([128, 3, 128], bf16, tag="pTs")
                nc.scalar.activation(pT, pT_ps, AF.Identity, bias=0.0,
                                     scale=1.0)

                # out_blk[i, d] = sum_j p[i, j] xrow[j, d]
                o_ps = psO.tile([128, D], f32, tag="o")
                for jt in range(3):
                    nc.tensor.matmul(o_ps, lhsT=pT[:, jt, :],
                                     rhs=xrowp[:, qb + jt, :],
                                     start=(jt == 0), stop=(jt == 2))
                o_sb = work.tile([128, D], bf16, tag="o_sb")
                nc.vector.tensor_scalar_mul(o_sb, o_ps, rinv)
                nc.sync.dma_start(out_d[qb * 128:(qb + 1) * 128, :], o_sb)

            for c, (c0, cw) in enumerate(CHUNKS):
                proj_chunk(c0, cw, c)
                for qb in CHUNK_BLOCKS[c]:
                    attention(qb)

    nc.compile()
    return nc


def _get_program():
    if "nc" not in _cached:
        _cached["nc"] = _build_program()
    return _cached["nc"]


def kernel(x, Wq_w, Wq_b, Wk_w, Wk_b, _trace=False):
    import ml_dtypes
    from concourse.bass_utils import run_bass_kernel_spmd

    bf16 = ml_dtypes.bfloat16

    x = np.ascontiguousarray(np.asarray(x, np.float32))
    Wq_w = np.asarray(Wq_w, np.float64)
    Wk_w = np.asarray(Wk_w, np.float64)
    # fold the projections: s_ij = x_i^T M x_j + beta_j (+ softmax-invariant
    # terms); M = Wq^T Wk, v = Wk^T bq
    M = (Wq_w.T @ Wk_w).astype(np.float32)
    v = (Wk_w.T @ np.asarray(Wq_b, np.float64)).astype(np.float32)
    # mtp[p, dti, m] = M[m, dti*128+p]
    mtp = np.ascontiguousarray(
        M.T.reshape(DT, 128, D).transpose(1, 0, 2))

    nc = _get_program()

    in_maps = []
    for core in range(N_CORES):
        b, h = divmod(core, 2)
        x_halo = np.zeros((SK, D), np.float32)
        if h == 0:
            x_halo[HALO:] = x[b, 0:HALF + HALO]
        else:
            x_halo[HALO:] = x[b, S - HALF - HALO:][::-1]
        # xTp[p, dt, c] = x_halo[c, dt*128+p], split into chunk tensors
        xTp = x_halo.T.reshape(DT, 128, SK).transpose(1, 0, 2)
        # xrowp[p, kt, d] = x_halo[kt*128+p, d]
        xrp = np.ascontiguousarray(
            x_halo.reshape(KT, 128, D).transpose(1, 0, 2)).astype(bf16)
        betar = (x_halo @ v).astype(bf16)
        im = {
            "mtp": mtp,
            "betap": np.ascontiguousarray(
                np.broadcast_to(betar[None, :], (128, SK))),
            "xrowp": xrp,
        }
        for c, (c0, cw) in enumerate(CHUNKS):
            im[f"xTp{c}"] = np.ascontiguousarray(xTp[:, :, c0:c0 + cw])
        in_maps.append(im)

    res = run_bass_kernel_spmd(nc, in_maps, core_ids=list(range(N_CORES)),
                               trace=_trace)
    _cached["last_result"] = res

    y = np.zeros((B, S, D), np.float32)
    for core in range(N_CORES):
        b, h = divmod(core, 2)
        o = res.results[core]["out"].astype(np.float32)
        if h == 0:
            y[b, :HALF] = o
        else:
            y[b, HALF:] = o[::-1]
    return y
